# revision 54
# baseline (speedup 1.0000x reference)
# BitAttention (ternary-quantized GQA transformer block) on 8 Trainium2 NeuronCores.
#
# Reference computation (see problem):
#   w_q = sign(w) * mean(|w|)            (per weight tensor, global scale)
#   q = x @ w_q(wq).T ; k = x @ w_q(wk).T ; v = x @ w_q(wv).T
#   GQA causal attention (32 q heads, 8 kv heads, head_dim 64)
#   out = attn @ w_q(wo).T
#
# Sharding (8 cores): batch (2) x kv-head-group (4).  Each core computes
# attention for 2 kv heads / 8 q heads of one batch and a partial out-proj
# over its 512 attention-output features; the host sums 4 partials per batch.
#
# Device layout: activations are kept feature-major ("transposed", [feat, token]).
# x enters pre-transposed in bf16; weights enter HOST-side sign-quantized to
# {+1,-1} in fp8e4 (exact), packed row-tiled so every weight DMA is one
# contiguous transfer.  The global quant scales enter as a tiny [1,2] fp32
# tensor and are folded into the softmax exp() scale (sq*sk/sqrt(hd)) and the
# V-transpose copy (sv*so).  V/O projections and all attention matmuls run
# bf16 (fp8 weights stream against bf16 activations at full rate); the Q
# projection runs fp8 DoubleRow (x cast to fp8e4 on device, 2 contraction
# tiles per matmul, ~2x) -- its quantization noise largely cancels in the
# softmax and costs ~1e-2 relative error, well inside the 2e-2 budget.
#
# Schedule: inputs stream over BOTH hardware DGE queues (sync + scalar);
# only the first 512-token block of the K/V/Q projections is computed before
# attention starts.  All remaining projection work (K/V token blocks, V
# transposes via DMA-XBAR, Q blocks, partial out-proj groups) is queued as
# PE "filler" items dripped ~one per attention chunk, with (qb, ft) deadlines
# force-emitting anything the next phase needs; out-proj groups drip in the
# late (ACT-paced, filler-starved) windows.  Scores for causal-diagonal key
# tiles are shortened to the live query sub-range (the masked-out prefix is
# never computed), which also shrinks the exp() and masking work; a single
# [128,128] triangle mask handles every diagonal.  Per-head-pair softmax
# normalization is deferred past the next pair's first chunk so the PE never
# head-of-line blocks on the DVE rowsum round-trip; its two K=1 broadcast
# matmuls sit on opposite PE row strips and run concurrently, and the final
# multiply writes o_sb directly with a partition-shifted DVE op (no staging
# DMA).  Softmax runs without max-subtraction (scores are O(1) by
# construction); the rowsum rides as a "ones" column appended to V;
# 1/rowsum is a fast DVE reciprocal.

import sys

for _p in ("/opt/trn_rl_repo",):
    if _p not in sys.path:
        sys.path.append(_p)

import numpy as np
import ml_dtypes

import concourse.bass as bass
import concourse.tile as tile
from concourse import bacc, mybir
from concourse import bass_utils
from concourse.masks import make_identity

F32 = mybir.dt.float32
BF16 = mybir.dt.bfloat16
FP8 = mybir.dt.float8e4
ALU = mybir.AluOpType
ACT = mybir.ActivationFunctionType
DR = mybir.MatmulPerfMode.DoubleRow

D = 2048          # model dim
S = 2048          # sequence length
B = 2             # batch
HD = 64           # head dim
NQH = 8           # q heads per core
NKV = 2           # kv heads per core
QF = NQH * HD     # 512 q features per core
KF = NKV * HD     # 128 kv features per core
QB = 512          # query block (free dim of score matmuls)
KT = 128          # key tile (partition dim of transposed scores)
NKT = S // KT     # 16
NQB = S // QB     # 4
NDT = D // 128    # 16 contraction tiles
EPS = 1e-5

# processing order of local q heads: tile ft holds heads (ft, ft+4) so that
# the head's row block (64*(h//4)) matches its kv head's row block in k_sb.
PERM = [0, 4, 1, 5, 2, 6, 3, 7]

_NC = None
_LAST_RESULTS = None


def _build():
    nc = bacc.Bacc("TRN2", target_bir_lowering=False, debug=False, num_devices=8)

    # weights arrive pre-packed host-side as [128, ...] row-tiled layouts so
    # every DMA is a single fully-contiguous transfer (4KB+ partition rows)
    xt_d = nc.dram_tensor("xt", [D, S], BF16, kind="ExternalInput")
    wqt_d = nc.dram_tensor("wqt", [128, 4 * NDT * 128], FP8, kind="ExternalInput")
    wkt_d = nc.dram_tensor("wkt", [128, NDT * KF], FP8, kind="ExternalInput")
    wvt_d = nc.dram_tensor("wvt", [128, NDT * KF], FP8, kind="ExternalInput")
    wot_d = nc.dram_tensor("wot", [128, 4 * D], FP8, kind="ExternalInput")
    sc_d = nc.dram_tensor("sc", [1, 2], F32, kind="ExternalInput")
    yt_d = nc.dram_tensor("yt", [D, S], BF16, kind="ExternalOutput")

    with tile.TileContext(nc) as tc:
        with (
            tc.tile_pool(name="persist", bufs=1) as pers,
            tc.tile_pool(name="stg", bufs=4) as stg,
            tc.tile_pool(name="work", bufs=3) as work,
            tc.tile_pool(name="exps_p", bufs=5) as exps_p,
            tc.tile_pool(name="ysb_p", bufs=8) as ysb_p,
            tc.tile_pool(name="mm", bufs=2, space="PSUM") as mm,
            tc.tile_pool(name="scp", bufs=2, space="PSUM") as scp,
            tc.tile_pool(name="pop", bufs=2, space="PSUM") as pop,
        ):
            # ---- constants ----
            sscore_bc = pers.tile([128, 1], F32, tag="sscore")
            sout_bc = pers.tile([128, 1], F32, tag="sout")
            nc.sync.dma_start(out=sscore_bc, in_=sc_d[0:1, 0:1].to_broadcast([128, 1]))
            nc.sync.dma_start(out=sout_bc, in_=sc_d[0:1, 1:2].to_broadcast([128, 1]))
            ident = pers.tile([128, 128], BF16, tag="ident")
            make_identity(nc, ident)
            # ones row at partition 64 for the rowsum-broadcast matmul
            ones64 = pers.tile([HD + 1, HD], F32, tag="ones64")
            nc.gpsimd.memset(ones64, 1.0)
            # triangle mask for the causal diagonal 128x128 sub-block:
            # tri[k, f] = 1.0 where f >= k else 0.0
            tri = pers.tile([128, KT], BF16, tag="tri")
            nc.gpsimd.memset(tri, 1.0)
            nc.gpsimd.affine_select(
                out=tri, in_=tri, compare_op=ALU.is_ge, fill=0.0,
                base=0, pattern=[[1, KT]], channel_multiplier=-1,
            )

            # ---- input streaming over both HW DGE queues ----
            # Weights arrive already sign-quantized ({+1,-1}) from the host,
            # packed so every DMA is one fully-contiguous transfer.  x tiles
            # stream FIRST on both queues (they gate every projection); small
            # weights follow, ft0's q-weights before the rest.
            queues = [nc.sync, nc.scalar]

            wk_all = pers.tile([128, NDT, KF], FP8, tag="wk")
            wv_all = pers.tile([128, NDT, KF], FP8, tag="wv")
            wq_all = pers.tile([128, 4, NDT, 128], FP8, tag="wq")
            wo_all = pers.tile([128, QF // 128, D], FP8, tag="wo")

            # fp8 copy of x feeds the DoubleRow q/k projections (2 contraction
            # tiles per matmul); bf16 x stays for the precision-sensitive V.
            # tiny critical weights first (~1us each) so the K/Q projections
            # can stream while x lands; bulk wq/wo after x.
            FT = NDT * 128
            nc.sync.dma_start(out=wk_all, in_=wkt_d[:, :])
            nc.sync.dma_start(out=wq_all[:, 0], in_=wqt_d[:, 0:FT])
            nc.scalar.dma_start(out=wv_all, in_=wvt_d[:, :])
            x8_all = pers.tile([128, NDT, S], FP8, tag="xq8")
            x_sb = []
            for t in range(NDT):
                xsb = pers.tile([128, S], BF16, tag=f"x{t}", name=f"x{t}")
                queues[t % 2].dma_start(out=xsb, in_=xt_d[t * 128:(t + 1) * 128, :])
                x_sb.append(xsb)
            for ft in range(1, 4):
                queues[ft % 2].dma_start(
                    out=wq_all[:, ft], in_=wqt_d[:, ft * FT:(ft + 1) * FT]
                )
            for h in range(2):
                nc.sync.dma_start(
                    out=wo_all[:, 2 * h:2 * h + 2, :],
                    in_=wot_d[:, 2 * h * D:(2 * h + 2) * D],
                )

            wv_sb = [wv_all[:, t, :] for t in range(NDT)]
            wk_sb = [wk_all[:, t, :] for t in range(NDT)]
            wo_sb = [wo_all[:, t, :] for t in range(QF // 128)]

            # ---- projections (feature-major: out[feat, token]) ----
            def proj_block(w_aps, out_sb, qb, use_scp=False, act_copy=False):
                # one 512-token block: out_sb[:, block] = sum_t w[t].T @ x[t]
                # startup blocks borrow the (idle) scores psum pool so K/V/Q
                # don't serialize on the 2-slot mm pool
                if use_scp:
                    ps2 = scp.tile([128, 2, QB], F32, tag="sc", name="startup_ps")
                    ps = ps2[:, 0, :]
                else:
                    ps = mm.tile([128, QB], F32, tag="mm")
                for t in range(NDT):
                    nc.tensor.matmul(
                        ps,
                        w_aps[t],
                        x_sb[t][:, qb * QB:(qb + 1) * QB],
                        start=(t == 0),
                        stop=(t == NDT - 1),
                    )
                if act_copy:
                    nc.scalar.activation(
                        out=out_sb[:, qb * QB:(qb + 1) * QB], in_=ps,
                        func=ACT.Copy, bias=0.0, scale=1.0,
                    )
                else:
                    nc.vector.tensor_copy(
                        out_sb[:, qb * QB:(qb + 1) * QB], ps
                    )

            def proj_block_dr(lhs_fn, out_sb, qb, use_scp=False):
                # fp8 DoubleRow projection: 2 contraction tiles per matmul
                if use_scp:
                    ps2 = scp.tile([128, 2, QB], F32, tag="sc", name="startup_ps_dr")
                    ps = ps2[:, 0, :]
                else:
                    ps = mm.tile([128, QB], F32, tag="mm")
                for j in range(NDT // 2):
                    nc.tensor.matmul(
                        ps,
                        lhs_fn(j),
                        x8_all[:, 2 * j:2 * j + 2, qb * QB:(qb + 1) * QB],
                        start=(j == 0),
                        stop=(j == NDT // 2 - 1),
                        perf_mode=DR,
                    )
                nc.vector.tensor_copy(
                    out_sb[:, qb * QB:(qb + 1) * QB], ps
                )

            def wk_pair(j):
                return wk_all[:, 2 * j:2 * j + 2, :]

            def wq_pair(ft):
                return lambda j: wq_all[:, ft, 2 * j:2 * j + 2, :]

            k_sb = pers.tile([128, S], BF16, tag="ksb")
            vf_sb = pers.tile([128, S], BF16, tag="vfsb")
            q_sb = [
                pers.tile([128, S], BF16, tag=f"qsb{ft}", name=f"qsb{ft}")
                for ft in range(4)
            ]
            o_sb = [
                pers.tile([128, S], BF16, tag=f"osb{ft}", name=f"osb{ft}")
                for ft in range(4)
            ]

            # token-major V with a trailing ones column (scaled by sv*so)
            vtok = [
                pers.tile([128, NKV, HD + 1], BF16, tag=f"vtok{t}", name=f"vtok{t}")
                for t in range(NKT)
            ]

            def emit_vtok(t):
                # token-major V via DMA-XBAR transpose (keeps the PE free)
                vt = vtok[t]
                tt = work.tile([128, 128], BF16, tag="vtt")
                nc.sync.dma_start(
                    out=tt, in_=vf_sb[:, t * 128:(t + 1) * 128], transpose=True
                )
                for kv in range(NKV):
                    nc.vector.tensor_scalar(
                        vt[:, kv, 0:HD], tt[:, kv * HD:(kv + 1) * HD],
                        sout_bc, None, ALU.mult,
                    )
                nc.vector.memset(vt[:, :, HD:HD + 1], 1.0)

            def emit_ygroup(qb, ot, qi=0, use_scp=False):
                # one partial out-projection psum group for query block qb;
                # the drain borrows the (dead) scores pool for deeper overlap
                q0 = qb * QB
                if use_scp:
                    py2 = scp.tile([128, 2, QB], F32, tag="sc", name="ydrain")
                    py = py2[:, 0, :]
                else:
                    py = mm.tile([128, QB], F32, tag="mm")
                for it in range(4):
                    nc.tensor.matmul(
                        py,
                        wo_sb[it][:, ot * 128:(ot + 1) * 128],
                        o_sb[it][:, q0:q0 + QB],
                        start=(it == 0),
                        stop=(it == 3),
                    )
                ysb = ysb_p.tile([128, QB], BF16, tag="ysb")
                nc.vector.tensor_copy(ysb, py)
                queues[qi].dma_start(
                    out=yt_d[ot * 128:(ot + 1) * 128, q0:q0 + QB], in_=ysb
                )

            # ---- PE filler queue ----
            # (deadline, fn, args): deadline (qb, ft) = must be emitted before
            # that attention phase begins.  Dripped ~1 item per chunk so the
            # PE never idles while ACT computes exps.
            END = (NQB, 0)
            filler = []

            def drip(n=1):
                for _ in range(n):
                    if filler:
                        _, fn, args = filler.pop(0)
                        fn(*args)

            def keepalive():
                # tiny matmul pulse so the HAM clock-gate stays at full rate
                # through ACT-paced stretches with no real PE work queued
                dmt = mm.tile([128, QB], F32, tag="mm", name="keepalive")
                nc.tensor.matmul(
                    dmt[0:HD, 0:HD], ones64[0:1, 0:HD], ones64[0:1, 0:HD],
                    start=True, stop=True,
                )

            def force(deadline):
                i = 0
                while i < len(filler):
                    if filler[i][0] <= deadline:
                        _, fn, args = filler.pop(i)
                        fn(*args)
                    else:
                        i += 1

            def push_qb_fillers(nqb, include_kv=True):
                # work needed for attention phase nqb (K/V/vtok/Q), pushed one
                # phase ahead so it drips during the current phase.
                dl = (nqb, 0)
                if include_kv:
                    filler.append((dl, proj_block, (wk_sb, k_sb, nqb)))
                    filler.append((dl, proj_block, (wv_sb, vf_sb, nqb)))
                    for t in range(4 * nqb, 4 * nqb + 4):
                        filler.append((dl, emit_vtok, (t,)))
                for ft in range(4):
                    filler.append(((nqb, ft), proj_block_dr, (wq_pair(ft), q_sb[ft], nqb)))

            # ---- startup: just enough projection work for (qb=0, ft=0) ----
            # K then Q first (they gate the first exp); V/vtok are only
            # needed one chunk later, so they follow in priority order.
            proj_block(wk_sb, k_sb, 0, use_scp=True, act_copy=True)
            # bf16 for the very first Q block: it must not wait on the x8
            # casts, which queue on DVE behind the startup copies
            wq0_sb = [wq_all[:, 0, t, :] for t in range(NDT)]
            proj_block(wq0_sb, q_sb[0], 0, act_copy=True)
            proj_block(wv_sb, vf_sb, 0, use_scp=True, act_copy=True)
            for t in range(4):
                emit_vtok(t)
            for t in range(NDT):
                nc.vector.tensor_copy(x8_all[:, t, :], x_sb[t])
            # bf16 K/V blocks for phase 1 lead the filler queue: the Q
            # DoubleRow fillers depend on the x8 casts, which are still
            # draining when the first chunks drip
            filler.append(((1, 0), proj_block, (wk_sb, k_sb, 1)))
            filler.append(((1, 0), proj_block, (wv_sb, vf_sb, 1)))
            for ft in range(1, 4):
                filler.append(((0, ft), proj_block_dr, (wq_pair(ft), q_sb[ft], 0)))

            pending_norm = None

            def emit_norm(po_, ft, q0):
                # normalize: O[:, q] * (1 / rowsum[q]); rowsum is po row 64.
                # The two heads' rowsum rows land on opposite partition strips
                # so their K=1 broadcast matmuls run concurrently; reciprocal
                # on DVE (approx_fast, ~4e-6 rel; rowsums well-conditioned);
                # the final multiply writes o_sb directly (partition-shifted
                # for the second head), no staging DMA.
                rs = work.tile([128, QB], F32, tag="rsum")
                nc.vector.tensor_copy(rs[HD:HD + 1, :], po_[0][HD:HD + 1, :])
                nc.vector.tensor_copy(rs[0:1, :], po_[1][HD:HD + 1, :])
                bcp = [None, None]
                bcp[0] = mm.tile([HD, QB], F32, tag="mm", name="bcp0")
                nc.tensor.matmul(
                    bcp[0], ones64[HD:HD + 1, :], rs[HD:HD + 1, :],
                    start=True, stop=True,
                )
                bcp[1] = mm.tile([HD, QB], F32, tag="mm", name="bcp1")
                nc.tensor.matmul(
                    bcp[1], ones64[0:1, :], rs[0:1, :],
                    start=True, stop=True,
                )
                for p in range(2):
                    bcr = work.tile([HD, QB], F32, tag="bcr")
                    nc.vector.reciprocal_approx_fast(out=bcr, in_=bcp[p])
                    nc.vector.tensor_tensor(
                        o_sb[ft][p * HD:(p + 1) * HD, q0:q0 + QB],
                        po_[p][0:HD, :], bcr, ALU.mult,
                    )

            # ---- attention ----
            # per (query block, q-tile): head pair (ft -> rows 0:64, ft+4 ->
            # rows 64:128); the two heads' score matmuls run concurrently on
            # different PE row strips; PV lags scores by one chunk so the PE
            # never waits on the ACT exp.
            for qb in range(NQB):
                q0 = qb * QB
                nkt = 4 * (qb + 1)          # causal: key tiles 0..nkt-1
                force((qb, 0))
                if qb == 0:
                    # K(1)/V(1) already lead the queue; vtok(1) follows the
                    # Q(0, ft*) fillers appended at startup
                    for t in range(4, 8):
                        filler.append(((1, 0), emit_vtok, (t,)))
                    push_qb_fillers(1, include_kv=False)
                elif qb + 1 < NQB:
                    push_qb_fillers(qb + 1)
                # out-proj groups drip in the LATE windows (which have the
                # most ACT-paced slack): y(0) during qb=2, y(1)+y(2) during
                # qb=3, y(3) in the drain.
                if qb == 2:
                    filler.extend(
                        (END, emit_ygroup, (0, ot)) for ot in range(NDT)
                    )
                if qb == 3:
                    for yqb in (1, 2):
                        filler.extend(
                            (END, emit_ygroup, (yqb, ot)) for ot in range(NDT)
                        )
                for ft in range(4):
                    force((qb, ft))
                    po_ = None  # allocated after the previous pair's deferred
                    # normalize is emitted, so the pool's WAR tracking sees
                    # those reads before the new tiles claim the slots.

                    def emit_pv(kt, ex, qoff):
                        for p in range(2):
                            nc.tensor.matmul(
                                po_[p][:, qoff:QB],
                                vtok[kt][:, p, :],
                                ex[:, p, qoff:QB],
                                start=(kt == 0),
                                stop=(kt == nkt - 1),
                            )

                    prev = None
                    for kt in range(nkt):
                        # diagonal tiles: queries before the tile's first key
                        # are fully masked -- skip those columns entirely.
                        dmi = kt - 4 * qb
                        qoff = max(dmi, 0) * KT
                        # both heads' scores for one key tile in a 2-bank psum
                        # tile; bufs=2 so the next tile's scores run on PE
                        # while ACT computes this tile's exp.
                        ps = scp.tile([128, 2, QB], F32, tag="sc", bufs=2,
                                      name=f"sc{qb}_{ft}_{kt}")
                        k0 = kt * KT
                        for p in range(2):
                            r0 = p * HD
                            nc.tensor.matmul(
                                ps[:, p, qoff:QB],
                                k_sb[r0:r0 + HD, k0:k0 + KT],
                                q_sb[ft][r0:r0 + HD, q0 + qoff:q0 + QB],
                                start=True, stop=True,
                            )
                        ex = exps_p.tile([128, 2, QB], BF16, tag="ex", bufs=4,
                                         name=f"ex{qb}_{ft}_{kt}")
                        nc.scalar.activation(
                            out=ex[:, :, qoff:QB], in_=ps[:, :, qoff:QB],
                            func=ACT.Exp, scale=sscore_bc,
                        )
                        if kt == 0:
                            # boundary fillers BEFORE the deferred normalize:
                            # its broadcast matmuls wait on a DVE round-trip
                            # and would head-of-line block the PE queue
                            drip(2)
                            if pending_norm is not None:
                                emit_norm(*pending_norm)
                                pending_norm = None
                            po_ = [
                                pop.tile([HD + 1, QB], F32, tag="po",
                                         name=f"po{qb}_{ft}_{p}")
                                for p in range(2)
                            ]
                        if dmi >= 0:  # diagonal tile: triangle-mask the first
                            for p in range(2):  # 128-query sub-block
                                nc.vector.tensor_tensor(
                                    ex[:, p, qoff:qoff + KT],
                                    ex[:, p, qoff:qoff + KT], tri, ALU.mult,
                                )
                        if prev is not None:
                            emit_pv(kt - 1, *prev)
                        prev = (ex, qoff)
                        # weight drips toward the head-pair boundary (where
                        # the chunk pipeline restarts and the PE is starved);
                        # in the last window, ration fillers so they stretch
                        # to the end instead of bunching early.
                        if kt == 0:
                            n = 0  # boundary drip already emitted above
                        elif kt == 1:
                            n = 2
                        elif qb == NQB - 1:
                            n = 1 if kt % 2 == 0 else 0
                        else:
                            n = 1
                        if qb <= 1 and len(filler) > 16:
                            n += 1
                        drip(n)
                        if qb >= 2 and kt < 3 and not filler:
                            keepalive()
                    emit_pv(nkt - 1, *prev)
                    pending_norm = (po_, ft, q0)

            # drain: last normalize, leftover fillers, last block's out-proj
            # (y DMAs alternate queues -- the scalar queue is idle by now)
            emit_norm(*pending_norm)
            force(END)
            # drain out-proj: batch 4 ot-groups into one 512KB DMA each so
            # the final writeback is 4 large transfers, not 16 small ones
            ybig = [
                pers.tile([128, 4, QB], BF16, tag=f"ybig{i}", name=f"ybig{i}")
                for i in range(2)
            ]
            q0d = (NQB - 1) * QB
            for ot in range(NDT):
                g, j = divmod(ot, 4)
                if ot % 2 == 1:
                    py2 = scp.tile([128, 2, QB], F32, tag="sc", name="ydrain")
                    py = py2[:, 0, :]
                else:
                    py = mm.tile([128, QB], F32, tag="mm")
                for it in range(4):
                    nc.tensor.matmul(
                        py,
                        wo_sb[it][:, ot * 128:(ot + 1) * 128],
                        o_sb[it][:, q0d:q0d + QB],
                        start=(it == 0),
                        stop=(it == 3),
                    )
                if ot % 2 == 0:
                    nc.vector.tensor_copy(ybig[g % 2][:, j, :], py)
                else:
                    nc.scalar.activation(
                        out=ybig[g % 2][:, j, :], in_=py, func=ACT.Copy,
                        bias=0.0, scale=1.0,
                    )
                if j == 3:
                    queues[g % 2].dma_start(
                        out=yt_d[4 * g * 128:(4 * g + 4) * 128, q0d:q0d + QB]
                        .rearrange("(j p) c -> p j c", p=128),
                        in_=ybig[g % 2],
                    )

    # The ACT table-set selector assigns Exp -> exp_and_others and
    # Ln -> natural_log (first set containing each func), which thrashes the
    # table RAM (~2.7us per switch) on every ln<->exp transition.  Both live
    # in natural_log_exp_and_others; steer the selector there by hiding
    # exp/ln from the other sets during this compile.
    import concourse.bacc as bacc_mod

    orig_tables = bacc_mod.get_activation_tables

    def one_set_tables(arch):
        t = orig_tables(arch)
        for name, fns in t.items():
            if name != "natural_log_exp_and_others":
                fns.discard(ACT.Exp)
                fns.discard(ACT.Ln)
        return t

    bacc_mod.get_activation_tables = one_set_tables
    try:
        nc.compile()
    finally:
        bacc_mod.get_activation_tables = orig_tables
    return nc


def _get_nc():
    global _NC
    if _NC is None:
        _NC = _build()
    return _NC


def run(inputs, trace=False, trace_cores=None):
    global _LAST_RESULTS
    x = np.asarray(inputs["x"], dtype=np.float32)
    wq = np.asarray(inputs["wq"], dtype=np.float32)
    wk = np.asarray(inputs["wk"], dtype=np.float32)
    wv = np.asarray(inputs["wv"], dtype=np.float32)
    wo = np.asarray(inputs["wo"], dtype=np.float32)

    sq = max(np.abs(wq).mean(), EPS)
    sk = max(np.abs(wk).mean(), EPS)
    sv = max(np.abs(wv).mean(), EPS)
    so = max(np.abs(wo).mean(), EPS)
    sc = np.array([[sq * sk / np.sqrt(HD), sv * so]], dtype=np.float32)

    perm_rows = np.concatenate([np.arange(h * HD, (h + 1) * HD) for h in PERM])

    def pack_rows(w):
        # [R, C] row-tiled to the device's [128, (R//128) * C] layout
        R, C = w.shape
        return np.ascontiguousarray(
            w.reshape(R // 128, 128, C).transpose(1, 0, 2).reshape(128, -1)
        )

    in_maps = []
    for c in range(8):
        b, g = divmod(c, 4)
        # device receives sign-quantized {+1,-1} weights (scales ride in sc)
        wq_g = np.sign(wq[QF * g:QF * (g + 1), :][perm_rows])     # [512, 2048]
        wk_g = np.sign(wk[KF * g:KF * (g + 1), :])                # [128, 2048]
        wv_g = np.sign(wv[KF * g:KF * (g + 1), :])
        wo_g = np.sign(wo[:, QF * g:QF * (g + 1)][:, perm_rows])  # [2048, 512]
        bf = ml_dtypes.bfloat16
        f8 = ml_dtypes.float8_e4m3
        # wq device layout: [p, ft, t, c] with d = t*128+p, col = ft*128+c
        wq_t = wq_g.T.reshape(NDT, 128, 4, 128).transpose(1, 2, 0, 3)
        in_maps.append({
            "xt": np.ascontiguousarray(x[b].T).astype(bf),
            "wqt": np.ascontiguousarray(wq_t.reshape(128, -1)).astype(f8),
            "wkt": pack_rows(wk_g.T).astype(f8),
            "wvt": pack_rows(wv_g.T).astype(f8),
            "wot": pack_rows(wo_g.T).astype(f8),
            "sc": sc,
        })

    nc = _get_nc()
    kwargs = {}
    if trace:
        kwargs["trace"] = True
        kwargs["trace_cores"] = trace_cores if trace_cores is not None else [0]
    res = bass_utils.run_bass_kernel_spmd(nc, in_maps, list(range(8)), **kwargs)
    _LAST_RESULTS = res

    y = np.empty((B, S, D), dtype=np.float32)
    for b in range(B):
        acc = np.zeros((D, S), dtype=np.float32)
        for g in range(4):
            acc += res.results[4 * b + g]["yt"].astype(np.float32)
        y[b] = acc.T
    return y


def kernel(**inputs):
    return run(inputs, trace=False)


# revision 55
# speedup vs baseline: 1.0328x; 1.0328x over previous
# BitAttention (ternary-quantized GQA transformer block) on 8 Trainium2 NeuronCores.
#
# Reference computation (see problem):
#   w_q = sign(w) * mean(|w|)            (per weight tensor, global scale)
#   q = x @ w_q(wq).T ; k = x @ w_q(wk).T ; v = x @ w_q(wv).T
#   GQA causal attention (32 q heads, 8 kv heads, head_dim 64)
#   out = attn @ w_q(wo).T
#
# Sharding (8 cores): batch (2) x kv-head-group (4).  Each core computes
# attention for 2 kv heads / 8 q heads of one batch and a partial out-proj
# over its 512 attention-output features; the host sums 4 partials per batch.
#
# Device layout: activations are kept feature-major ("transposed", [feat, token]).
# x enters pre-transposed in bf16; weights enter HOST-side sign-quantized to
# {+1,-1} in fp8e4 (exact), packed row-tiled so every weight DMA is one
# contiguous transfer.  The global quant scales enter as a tiny [1,2] fp32
# tensor and are folded into the softmax exp() scale (sq*sk/sqrt(hd)) and the
# V-transpose copy (sv*so).  V/O projections and all attention matmuls run
# bf16 (fp8 weights stream against bf16 activations at full rate); the Q
# projection runs fp8 DoubleRow (x cast to fp8e4 on device, 2 contraction
# tiles per matmul, ~2x) -- its quantization noise largely cancels in the
# softmax and costs ~1e-2 relative error, well inside the 2e-2 budget.
#
# Schedule: inputs stream over BOTH hardware DGE queues (sync + scalar);
# only the first 512-token block of the K/V/Q projections is computed before
# attention starts.  All remaining projection work (K/V token blocks, V
# transposes via DMA-XBAR, Q blocks, partial out-proj groups) is queued as
# PE "filler" items dripped ~one per attention chunk, with (qb, ft) deadlines
# force-emitting anything the next phase needs; out-proj groups drip in the
# late (ACT-paced, filler-starved) windows.  Scores for causal-diagonal key
# tiles are shortened to the live query sub-range (the masked-out prefix is
# never computed), which also shrinks the exp() and masking work; a single
# [128,128] triangle mask handles every diagonal.  Per-head-pair softmax
# normalization is deferred past the next pair's first chunk so the PE never
# head-of-line blocks on the DVE rowsum round-trip; its two K=1 broadcast
# matmuls sit on opposite PE row strips and run concurrently, and the final
# multiply writes o_sb directly with a partition-shifted DVE op (no staging
# DMA).  Softmax runs without max-subtraction (scores are O(1) by
# construction); the rowsum rides as a "ones" column appended to V;
# 1/rowsum is a fast DVE reciprocal.

import sys

for _p in ("/opt/trn_rl_repo",):
    if _p not in sys.path:
        sys.path.append(_p)

import numpy as np
import ml_dtypes

import concourse.bass as bass
import concourse.tile as tile
from concourse import bacc, mybir
from concourse import bass_utils
from concourse.masks import make_identity

F32 = mybir.dt.float32
BF16 = mybir.dt.bfloat16
FP8 = mybir.dt.float8e4
ALU = mybir.AluOpType
ACT = mybir.ActivationFunctionType
DR = mybir.MatmulPerfMode.DoubleRow

D = 2048          # model dim
S = 2048          # sequence length
B = 2             # batch
HD = 64           # head dim
NQH = 8           # q heads per core
NKV = 2           # kv heads per core
QF = NQH * HD     # 512 q features per core
KF = NKV * HD     # 128 kv features per core
QB = 512          # query block (free dim of score matmuls)
KT = 128          # key tile (partition dim of transposed scores)
NKT = S // KT     # 16
NQB = S // QB     # 4
NDT = D // 128    # 16 contraction tiles
EPS = 1e-5

# processing order of local q heads: tile ft holds heads (ft, ft+4) so that
# the head's row block (64*(h//4)) matches its kv head's row block in k_sb.
PERM = [0, 4, 1, 5, 2, 6, 3, 7]

_NC = None
_LAST_RESULTS = None


def _build():
    nc = bacc.Bacc("TRN2", target_bir_lowering=False, debug=False, num_devices=8)

    # weights arrive pre-packed host-side as [128, ...] row-tiled layouts so
    # every DMA is a single fully-contiguous transfer (4KB+ partition rows)
    xt_d = nc.dram_tensor("xt", [D, S], BF16, kind="ExternalInput")
    wqt_d = nc.dram_tensor("wqt", [128, 4 * NDT * 128], FP8, kind="ExternalInput")
    wkt_d = nc.dram_tensor("wkt", [128, NDT * KF], FP8, kind="ExternalInput")
    wvt_d = nc.dram_tensor("wvt", [128, NDT * KF], FP8, kind="ExternalInput")
    wot_d = nc.dram_tensor("wot", [128, 4 * D], FP8, kind="ExternalInput")
    sc_d = nc.dram_tensor("sc", [1, 2], F32, kind="ExternalInput")
    yt_d = nc.dram_tensor("yt", [D, S], BF16, kind="ExternalOutput")

    with tile.TileContext(nc) as tc:
        with (
            tc.tile_pool(name="persist", bufs=1) as pers,
            tc.tile_pool(name="stg", bufs=4) as stg,
            tc.tile_pool(name="work", bufs=3) as work,
            tc.tile_pool(name="exps_p", bufs=5) as exps_p,
            tc.tile_pool(name="ysb_p", bufs=8) as ysb_p,
            tc.tile_pool(name="mm", bufs=2, space="PSUM") as mm,
            tc.tile_pool(name="scp", bufs=2, space="PSUM") as scp,
            tc.tile_pool(name="pop", bufs=2, space="PSUM") as pop,
        ):
            # ---- constants ----
            sscore_bc = pers.tile([128, 1], F32, tag="sscore")
            sout_bc = pers.tile([128, 1], F32, tag="sout")
            nc.sync.dma_start(out=sscore_bc, in_=sc_d[0:1, 0:1].to_broadcast([128, 1]))
            nc.sync.dma_start(out=sout_bc, in_=sc_d[0:1, 1:2].to_broadcast([128, 1]))
            ident = pers.tile([128, 128], BF16, tag="ident")
            make_identity(nc, ident)
            # ones row at partition 64 for the rowsum-broadcast matmul
            ones64 = pers.tile([HD + 1, HD], F32, tag="ones64")
            nc.gpsimd.memset(ones64, 1.0)
            # triangle mask for the causal diagonal 128x128 sub-block:
            # tri[k, f] = 1.0 where f >= k else 0.0
            tri = pers.tile([128, KT], BF16, tag="tri")
            nc.gpsimd.memset(tri, 1.0)
            nc.gpsimd.affine_select(
                out=tri, in_=tri, compare_op=ALU.is_ge, fill=0.0,
                base=0, pattern=[[1, KT]], channel_multiplier=-1,
            )

            # ---- input streaming over both HW DGE queues ----
            # Weights arrive already sign-quantized ({+1,-1}) from the host,
            # packed so every DMA is one fully-contiguous transfer.  x tiles
            # stream FIRST on both queues (they gate every projection); small
            # weights follow, ft0's q-weights before the rest.
            queues = [nc.sync, nc.scalar]

            wk_all = pers.tile([128, NDT, KF], FP8, tag="wk")
            wv_all = pers.tile([128, NDT, KF], FP8, tag="wv")
            wq_all = pers.tile([128, 4, NDT, 128], FP8, tag="wq")
            wo_all = pers.tile([128, QF // 128, D], FP8, tag="wo")

            # fp8 copy of x feeds the DoubleRow q/k projections (2 contraction
            # tiles per matmul); bf16 x stays for the precision-sensitive V.
            # tiny critical weights first (~1us each) so the K/Q projections
            # can stream while x lands; bulk wq/wo after x.
            FT = NDT * 128
            nc.sync.dma_start(out=wk_all, in_=wkt_d[:, :])
            nc.sync.dma_start(out=wq_all[:, 0], in_=wqt_d[:, 0:FT])
            nc.scalar.dma_start(out=wv_all, in_=wvt_d[:, :])
            x8_all = pers.tile([128, NDT, S], FP8, tag="xq8")
            x_sb = []
            for t in range(NDT):
                xsb = pers.tile([128, S], BF16, tag=f"x{t}", name=f"x{t}")
                queues[t % 2].dma_start(out=xsb, in_=xt_d[t * 128:(t + 1) * 128, :])
                x_sb.append(xsb)
            for ft in range(1, 4):
                queues[ft % 2].dma_start(
                    out=wq_all[:, ft], in_=wqt_d[:, ft * FT:(ft + 1) * FT]
                )
            for h in range(2):
                nc.sync.dma_start(
                    out=wo_all[:, 2 * h:2 * h + 2, :],
                    in_=wot_d[:, 2 * h * D:(2 * h + 2) * D],
                )

            wv_sb = [wv_all[:, t, :] for t in range(NDT)]
            wk_sb = [wk_all[:, t, :] for t in range(NDT)]
            wo_sb = [wo_all[:, t, :] for t in range(QF // 128)]

            # ---- projections (feature-major: out[feat, token]) ----
            def proj_block(w_aps, out_sb, qb, use_scp=False):
                # one 512-token block: out_sb[:, block] = sum_t w[t].T @ x[t]
                # startup blocks borrow the (idle) scores psum pool so K/V/Q
                # don't serialize on the 2-slot mm pool
                if use_scp:
                    ps2 = scp.tile([128, 2, QB], F32, tag="sc", name="startup_ps")
                    ps = ps2[:, 0, :]
                else:
                    ps = mm.tile([128, QB], F32, tag="mm")
                for t in range(NDT):
                    nc.tensor.matmul(
                        ps,
                        w_aps[t],
                        x_sb[t][:, qb * QB:(qb + 1) * QB],
                        start=(t == 0),
                        stop=(t == NDT - 1),
                    )
                nc.vector.tensor_copy(
                    out_sb[:, qb * QB:(qb + 1) * QB], ps
                )

            def proj_block_dr(lhs_fn, out_sb, qb, use_scp=False):
                # fp8 DoubleRow projection: 2 contraction tiles per matmul
                if use_scp:
                    ps2 = scp.tile([128, 2, QB], F32, tag="sc", name="startup_ps_dr")
                    ps = ps2[:, 0, :]
                else:
                    ps = mm.tile([128, QB], F32, tag="mm")
                for j in range(NDT // 2):
                    nc.tensor.matmul(
                        ps,
                        lhs_fn(j),
                        x8_all[:, 2 * j:2 * j + 2, qb * QB:(qb + 1) * QB],
                        start=(j == 0),
                        stop=(j == NDT // 2 - 1),
                        perf_mode=DR,
                    )
                nc.vector.tensor_copy(
                    out_sb[:, qb * QB:(qb + 1) * QB], ps
                )

            def wk_pair(j):
                return wk_all[:, 2 * j:2 * j + 2, :]

            def wq_pair(ft):
                return lambda j: wq_all[:, ft, 2 * j:2 * j + 2, :]

            k_sb = pers.tile([128, S], BF16, tag="ksb")
            vf_sb = pers.tile([128, S], BF16, tag="vfsb")
            q_sb = [
                pers.tile([128, S], BF16, tag=f"qsb{ft}", name=f"qsb{ft}")
                for ft in range(4)
            ]
            o_sb = [
                pers.tile([128, S], BF16, tag=f"osb{ft}", name=f"osb{ft}")
                for ft in range(4)
            ]

            # token-major V with a trailing ones column (scaled by sv*so)
            vtok = [
                pers.tile([128, NKV, HD + 1], BF16, tag=f"vtok{t}", name=f"vtok{t}")
                for t in range(NKT)
            ]

            def emit_vtok(t):
                # token-major V via DMA-XBAR transpose (keeps the PE free)
                vt = vtok[t]
                tt = work.tile([128, 128], BF16, tag="vtt")
                nc.sync.dma_start(
                    out=tt, in_=vf_sb[:, t * 128:(t + 1) * 128], transpose=True
                )
                for kv in range(NKV):
                    nc.vector.tensor_scalar(
                        vt[:, kv, 0:HD], tt[:, kv * HD:(kv + 1) * HD],
                        sout_bc, None, ALU.mult,
                    )
                nc.vector.memset(vt[:, :, HD:HD + 1], 1.0)

            def emit_ygroup(qb, ot, qi=0, use_scp=False):
                # one partial out-projection psum group for query block qb;
                # the drain borrows the (dead) scores pool for deeper overlap
                q0 = qb * QB
                if use_scp:
                    py2 = scp.tile([128, 2, QB], F32, tag="sc", name="ydrain")
                    py = py2[:, 0, :]
                else:
                    py = mm.tile([128, QB], F32, tag="mm")
                for it in range(4):
                    nc.tensor.matmul(
                        py,
                        wo_sb[it][:, ot * 128:(ot + 1) * 128],
                        o_sb[it][:, q0:q0 + QB],
                        start=(it == 0),
                        stop=(it == 3),
                    )
                ysb = ysb_p.tile([128, QB], BF16, tag="ysb")
                nc.vector.tensor_copy(ysb, py)
                queues[qi].dma_start(
                    out=yt_d[ot * 128:(ot + 1) * 128, q0:q0 + QB], in_=ysb
                )

            # ---- PE filler queue ----
            # (deadline, fn, args): deadline (qb, ft) = must be emitted before
            # that attention phase begins.  Dripped ~1 item per chunk so the
            # PE never idles while ACT computes exps.
            END = (NQB, 0)
            filler = []

            def drip(n=1):
                for _ in range(n):
                    if filler:
                        _, fn, args = filler.pop(0)
                        fn(*args)

            def keepalive():
                # tiny matmul pulse so the HAM clock-gate stays at full rate
                # through ACT-paced stretches with no real PE work queued
                dmt = mm.tile([128, QB], F32, tag="mm", name="keepalive")
                nc.tensor.matmul(
                    dmt[0:HD, 0:HD], ones64[0:1, 0:HD], ones64[0:1, 0:HD],
                    start=True, stop=True,
                )

            def force(deadline):
                i = 0
                while i < len(filler):
                    if filler[i][0] <= deadline:
                        _, fn, args = filler.pop(i)
                        fn(*args)
                    else:
                        i += 1

            def push_qb_fillers(nqb, include_kv=True):
                # work needed for attention phase nqb (K/V/vtok/Q), pushed one
                # phase ahead so it drips during the current phase.
                dl = (nqb, 0)
                if include_kv:
                    filler.append((dl, proj_block, (wk_sb, k_sb, nqb)))
                    filler.append((dl, proj_block, (wv_sb, vf_sb, nqb)))
                    for t in range(4 * nqb, 4 * nqb + 4):
                        filler.append((dl, emit_vtok, (t,)))
                for ft in range(4):
                    filler.append(((nqb, ft), proj_block_dr, (wq_pair(ft), q_sb[ft], nqb)))

            # ---- startup: just enough projection work for (qb=0, ft=0) ----
            # K then Q first (they gate the first exp); V/vtok are only
            # needed one chunk later, so they follow in priority order.
            proj_block(wk_sb, k_sb, 0, use_scp=True)
            # bf16 for the very first Q block: it must not wait on the x8
            # casts, which queue on DVE behind the startup copies
            wq0_sb = [wq_all[:, 0, t, :] for t in range(NDT)]
            proj_block(wq0_sb, q_sb[0], 0)
            proj_block(wv_sb, vf_sb, 0, use_scp=True)
            for t in range(4):
                emit_vtok(t)
            for t in range(NDT):
                nc.vector.tensor_copy(x8_all[:, t, :], x_sb[t])
            # bf16 K/V blocks for phase 1 lead the filler queue: the Q
            # DoubleRow fillers depend on the x8 casts, which are still
            # draining when the first chunks drip
            filler.append(((1, 0), proj_block, (wk_sb, k_sb, 1)))
            filler.append(((1, 0), proj_block, (wv_sb, vf_sb, 1)))
            for ft in range(1, 4):
                filler.append(((0, ft), proj_block_dr, (wq_pair(ft), q_sb[ft], 0)))

            pending_norm = None

            def emit_norm(po_, ft, q0):
                # normalize: O[:, q] * (1 / rowsum[q]); rowsum is po row 64.
                # The two heads' rowsum rows land on opposite partition strips
                # so their K=1 broadcast matmuls run concurrently; reciprocal
                # on DVE (approx_fast, ~4e-6 rel; rowsums well-conditioned);
                # the final multiply writes o_sb directly (partition-shifted
                # for the second head), no staging DMA.
                rs = work.tile([128, QB], F32, tag="rsum")
                nc.vector.tensor_copy(rs[HD:HD + 1, :], po_[0][HD:HD + 1, :])
                nc.vector.tensor_copy(rs[0:1, :], po_[1][HD:HD + 1, :])
                bcp = [None, None]
                bcp[0] = mm.tile([HD, QB], F32, tag="mm", name="bcp0")
                nc.tensor.matmul(
                    bcp[0], ones64[HD:HD + 1, :], rs[HD:HD + 1, :],
                    start=True, stop=True,
                )
                bcp[1] = mm.tile([HD, QB], F32, tag="mm", name="bcp1")
                nc.tensor.matmul(
                    bcp[1], ones64[0:1, :], rs[0:1, :],
                    start=True, stop=True,
                )
                for p in range(2):
                    bcr = work.tile([HD, QB], F32, tag="bcr")
                    nc.vector.reciprocal_approx_fast(out=bcr, in_=bcp[p])
                    nc.vector.tensor_tensor(
                        o_sb[ft][p * HD:(p + 1) * HD, q0:q0 + QB],
                        po_[p][0:HD, :], bcr, ALU.mult,
                    )

            # ---- attention ----
            # per (query block, q-tile): head pair (ft -> rows 0:64, ft+4 ->
            # rows 64:128); the two heads' score matmuls run concurrently on
            # different PE row strips; PV lags scores by one chunk so the PE
            # never waits on the ACT exp.
            for qb in range(NQB):
                q0 = qb * QB
                nkt = 4 * (qb + 1)          # causal: key tiles 0..nkt-1
                force((qb, 0))
                if qb == 0:
                    # K(1)/V(1) already lead the queue; vtok(1) follows the
                    # Q(0, ft*) fillers appended at startup
                    for t in range(4, 8):
                        filler.append(((1, 0), emit_vtok, (t,)))
                    push_qb_fillers(1, include_kv=False)
                elif qb + 1 < NQB:
                    push_qb_fillers(qb + 1)
                # out-proj groups drip in the LATE windows (which have the
                # most ACT-paced slack): y(0) during qb=2, y(1)+y(2) during
                # qb=3, y(3) in the drain.
                if qb == 2:
                    filler.extend(
                        (END, emit_ygroup, (0, ot)) for ot in range(NDT)
                    )
                if qb == 3:
                    for yqb in (1, 2):
                        filler.extend(
                            (END, emit_ygroup, (yqb, ot)) for ot in range(NDT)
                        )
                for ft in range(4):
                    force((qb, ft))
                    po_ = None  # allocated after the previous pair's deferred
                    # normalize is emitted, so the pool's WAR tracking sees
                    # those reads before the new tiles claim the slots.

                    def emit_pv(kt, ex, qoff):
                        for p in range(2):
                            nc.tensor.matmul(
                                po_[p][:, qoff:QB],
                                vtok[kt][:, p, :],
                                ex[:, p, qoff:QB],
                                start=(kt == 0),
                                stop=(kt == nkt - 1),
                            )

                    prev = None
                    for kt in range(nkt):
                        # diagonal tiles: queries before the tile's first key
                        # are fully masked -- skip those columns entirely.
                        dmi = kt - 4 * qb
                        qoff = max(dmi, 0) * KT
                        # both heads' scores for one key tile in a 2-bank psum
                        # tile; bufs=2 so the next tile's scores run on PE
                        # while ACT computes this tile's exp.
                        ps = scp.tile([128, 2, QB], F32, tag="sc", bufs=2,
                                      name=f"sc{qb}_{ft}_{kt}")
                        k0 = kt * KT
                        for p in range(2):
                            r0 = p * HD
                            nc.tensor.matmul(
                                ps[:, p, qoff:QB],
                                k_sb[r0:r0 + HD, k0:k0 + KT],
                                q_sb[ft][r0:r0 + HD, q0 + qoff:q0 + QB],
                                start=True, stop=True,
                            )
                        ex = exps_p.tile([128, 2, QB], BF16, tag="ex", bufs=4,
                                         name=f"ex{qb}_{ft}_{kt}")
                        nc.scalar.activation(
                            out=ex[:, :, qoff:QB], in_=ps[:, :, qoff:QB],
                            func=ACT.Exp, scale=sscore_bc,
                        )
                        if kt == 0:
                            # boundary fillers BEFORE the deferred normalize:
                            # its broadcast matmuls wait on a DVE round-trip
                            # and would head-of-line block the PE queue
                            drip(2)
                            if pending_norm is not None:
                                emit_norm(*pending_norm)
                                pending_norm = None
                            po_ = [
                                pop.tile([HD + 1, QB], F32, tag="po",
                                         name=f"po{qb}_{ft}_{p}")
                                for p in range(2)
                            ]
                        if dmi >= 0:  # diagonal tile: triangle-mask the first
                            for p in range(2):  # 128-query sub-block
                                nc.vector.tensor_tensor(
                                    ex[:, p, qoff:qoff + KT],
                                    ex[:, p, qoff:qoff + KT], tri, ALU.mult,
                                )
                        if prev is not None:
                            emit_pv(kt - 1, *prev)
                        prev = (ex, qoff)
                        # weight drips toward the head-pair boundary (where
                        # the chunk pipeline restarts and the PE is starved);
                        # in the last window, ration fillers so they stretch
                        # to the end instead of bunching early.
                        if kt == 0:
                            n = 0  # boundary drip already emitted above
                        elif kt == 1:
                            n = 2
                        elif qb == NQB - 1:
                            n = 1 if kt % 2 == 0 else 0
                        else:
                            n = 1
                        if qb <= 1 and len(filler) > 16:
                            n += 1
                        drip(n)
                        if qb >= 2 and kt < 3 and not filler:
                            keepalive()
                    emit_pv(nkt - 1, *prev)
                    pending_norm = (po_, ft, q0)

            # drain: last normalize, leftover fillers, last block's out-proj
            # (y DMAs alternate queues -- the scalar queue is idle by now)
            emit_norm(*pending_norm)
            force(END)
            # drain out-proj: batch 4 ot-groups into one 512KB DMA each so
            # the final writeback is 4 large transfers, not 16 small ones
            ybig = [
                pers.tile([128, 4, QB], BF16, tag=f"ybig{i}", name=f"ybig{i}")
                for i in range(2)
            ]
            q0d = (NQB - 1) * QB
            for ot in range(NDT):
                g, j = divmod(ot, 4)
                if ot % 2 == 1:
                    py2 = scp.tile([128, 2, QB], F32, tag="sc", name="ydrain")
                    py = py2[:, 0, :]
                else:
                    py = mm.tile([128, QB], F32, tag="mm")
                for it in range(4):
                    nc.tensor.matmul(
                        py,
                        wo_sb[it][:, ot * 128:(ot + 1) * 128],
                        o_sb[it][:, q0d:q0d + QB],
                        start=(it == 0),
                        stop=(it == 3),
                    )
                nc.vector.tensor_copy(ybig[g % 2][:, j, :], py)
                if j == 3:
                    queues[g % 2].dma_start(
                        out=yt_d[4 * g * 128:(4 * g + 4) * 128, q0d:q0d + QB]
                        .rearrange("(j p) c -> p j c", p=128),
                        in_=ybig[g % 2],
                    )

    # The ACT table-set selector assigns Exp -> exp_and_others and
    # Ln -> natural_log (first set containing each func), which thrashes the
    # table RAM (~2.7us per switch) on every ln<->exp transition.  Both live
    # in natural_log_exp_and_others; steer the selector there by hiding
    # exp/ln from the other sets during this compile.
    import concourse.bacc as bacc_mod

    orig_tables = bacc_mod.get_activation_tables

    def one_set_tables(arch):
        t = orig_tables(arch)
        for name, fns in t.items():
            if name != "natural_log_exp_and_others":
                fns.discard(ACT.Exp)
                fns.discard(ACT.Ln)
        return t

    bacc_mod.get_activation_tables = one_set_tables
    try:
        nc.compile()
    finally:
        bacc_mod.get_activation_tables = orig_tables
    return nc


def _get_nc():
    global _NC
    if _NC is None:
        _NC = _build()
    return _NC


def run(inputs, trace=False, trace_cores=None):
    global _LAST_RESULTS
    x = np.asarray(inputs["x"], dtype=np.float32)
    wq = np.asarray(inputs["wq"], dtype=np.float32)
    wk = np.asarray(inputs["wk"], dtype=np.float32)
    wv = np.asarray(inputs["wv"], dtype=np.float32)
    wo = np.asarray(inputs["wo"], dtype=np.float32)

    sq = max(np.abs(wq).mean(), EPS)
    sk = max(np.abs(wk).mean(), EPS)
    sv = max(np.abs(wv).mean(), EPS)
    so = max(np.abs(wo).mean(), EPS)
    sc = np.array([[sq * sk / np.sqrt(HD), sv * so]], dtype=np.float32)

    perm_rows = np.concatenate([np.arange(h * HD, (h + 1) * HD) for h in PERM])

    def pack_rows(w):
        # [R, C] row-tiled to the device's [128, (R//128) * C] layout
        R, C = w.shape
        return np.ascontiguousarray(
            w.reshape(R // 128, 128, C).transpose(1, 0, 2).reshape(128, -1)
        )

    in_maps = []
    for c in range(8):
        b, g = divmod(c, 4)
        # device receives sign-quantized {+1,-1} weights (scales ride in sc)
        wq_g = np.sign(wq[QF * g:QF * (g + 1), :][perm_rows])     # [512, 2048]
        wk_g = np.sign(wk[KF * g:KF * (g + 1), :])                # [128, 2048]
        wv_g = np.sign(wv[KF * g:KF * (g + 1), :])
        wo_g = np.sign(wo[:, QF * g:QF * (g + 1)][:, perm_rows])  # [2048, 512]
        bf = ml_dtypes.bfloat16
        f8 = ml_dtypes.float8_e4m3
        # wq device layout: [p, ft, t, c] with d = t*128+p, col = ft*128+c
        wq_t = wq_g.T.reshape(NDT, 128, 4, 128).transpose(1, 2, 0, 3)
        in_maps.append({
            "xt": np.ascontiguousarray(x[b].T).astype(bf),
            "wqt": np.ascontiguousarray(wq_t.reshape(128, -1)).astype(f8),
            "wkt": pack_rows(wk_g.T).astype(f8),
            "wvt": pack_rows(wv_g.T).astype(f8),
            "wot": pack_rows(wo_g.T).astype(f8),
            "sc": sc,
        })

    nc = _get_nc()
    kwargs = {}
    if trace:
        kwargs["trace"] = True
        kwargs["trace_cores"] = trace_cores if trace_cores is not None else [0]
    res = bass_utils.run_bass_kernel_spmd(nc, in_maps, list(range(8)), **kwargs)
    _LAST_RESULTS = res

    y = np.empty((B, S, D), dtype=np.float32)
    for b in range(B):
        acc = np.zeros((D, S), dtype=np.float32)
        for g in range(4):
            acc += res.results[4 * b + g]["yt"].astype(np.float32)
        y[b] = acc.T
    return y


def kernel(**inputs):
    return run(inputs, trace=False)
